# revision 6
# baseline (speedup 1.0000x reference)
"""Trainium2 Bass kernel for nn_MultiHeadAttention (B=2, S=4096, D=1024, H=16, Dh=64).

Sharding over 8 cores: core c handles batch b=c//4 and head-group hg=c%4
(4 heads = 256 channels). Host gathers by summing the 4 per-head-group partial
output projections per batch (row-parallel output projection).

v2 design (PE p-state + engine-balance optimized):
  - Heads processed SEQUENTIALLY per (q-block, pair): each head's 32-kt score/
    exp/AV pipeline runs with double-buffered score PSUM, so the PE never
    bubbles and stays ramped at 2.4 GHz.
  - V||ones: Vsb stores per-head [V(64ch) | ones(64)] as the AV stationary,
    so every AV matmul also accumulates the softmax row-sum broadcast into
    PSUM rows 64..127 at zero extra PE cost (matmul cost = output columns).
    This deletes the baseline's DVE row-sum tree + closing matmuls that
    serialized each q-block and reset the PE p-state ramp.
  - Exp split across engines by kt parity: even kt on ACT (Exp, scale=1/8),
    odd kt on DVE via pow(e^{1/8}, s).  Neither engine exceeds the PE's
    per-kt budget, so exp is fully hidden.
  - Normalization: reciprocal_approx_fast on the rowsum rows + one DVE mult.
  - Q/K projection bias adds moved to ACT (idle during projections).

PSUM: score/out-proj pool 2x[128,1024]f32 (4 banks) + attn pool 2x[128,1024]
(4 banks) = 8 banks exactly.
"""

import math
import os
import sys
import functools

import numpy as np
import ml_dtypes

sys.path.insert(0, "/opt/trn_rl_repo")

import concourse.bass as bass  # noqa: E402
import concourse.mybir as mybir  # noqa: E402
import concourse.tile as tile  # noqa: E402
from concourse import bass_utils  # noqa: E402

B, S, D, H, DH = 2, 4096, 1024, 16, 64
NCORES = 8
HG = 4  # head groups (cores per batch)
OC = 256  # q/k/v channels per core
BF16 = mybir.dt.bfloat16
F32 = mybir.dt.float32
QBLK = 1024
NQB = S // QBLK  # 4
NKT = S // 128  # 32 k-tiles
bf16 = ml_dtypes.bfloat16
EXP_SCALE = 1.0 / math.sqrt(DH)
EBASE = math.exp(EXP_SCALE)  # pow(EBASE, s) == exp(s/sqrt(DH))


_TPB_ENGINES = None


def _split_waits(nc, max_waits=1):
    """walrus codegen in this container rejects TPB instructions carrying more
    than one sync-wait command.  Spill extra semaphore waits onto preceding
    NoOps on the same engine (engines execute their queue in order, so a NoOp
    that waits immediately before the instruction is equivalent)."""
    import bass_rust

    global _TPB_ENGINES
    if _TPB_ENGINES is None:
        _TPB_ENGINES = {
            mybir.EngineType.Pool,
            mybir.EngineType.Activation,
            mybir.EngineType.PE,
            mybir.EngineType.DVE,
            mybir.EngineType.SP,
        }
    ctr = 0
    for bb in nc.main_func.blocks:
        insts = bb.instructions
        out = []
        changed = False
        for inst in insts:
            si = getattr(inst, "sync_info", None)
            if (
                si is not None
                and si.on_wait
                and len(si.on_wait) > max_waits
                and inst.engine in _TPB_ENGINES
            ):
                waits = list(si.on_wait)
                keep = waits[-max_waits:]
                spill = waits[:-max_waits]
                for i in range(0, len(spill), max_waits):
                    nop = bass_rust.InstNoOp(
                        name=f"{inst.name}-sw{ctr}", ins=[], outs=[]
                    )
                    ctr += 1
                    nop.engine = inst.engine
                    nop.sync_info = mybir.SyncInfo(
                        on_wait=spill[i : i + max_waits], on_update=[]
                    )
                    out.append(nop)
                inst.sync_info = mybir.SyncInfo(
                    on_wait=keep, on_update=list(si.on_update)
                )
                changed = True
            out.append(inst)
        if changed:
            insts[:] = out
    return nc


@functools.lru_cache(maxsize=4)
def _build(masked: bool, split_waits: bool = True):
    nc = bass.Bass()

    xqT_d = nc.dram_tensor("xqT", [D, S], BF16, kind="ExternalInput")
    xkT_d = nc.dram_tensor("xkT", [D, S], BF16, kind="ExternalInput")
    xvT_d = nc.dram_tensor("xvT", [D, S], BF16, kind="ExternalInput")
    wqT_d = nc.dram_tensor("wqT", [D, OC], BF16, kind="ExternalInput")
    wkT_d = nc.dram_tensor("wkT", [D, OC], BF16, kind="ExternalInput")
    wvT_d = nc.dram_tensor("wvT", [D, OC], BF16, kind="ExternalInput")
    bq_d = nc.dram_tensor("bq2", [128, 2], F32, kind="ExternalInput")
    bk_d = nc.dram_tensor("bk2", [128, 2], F32, kind="ExternalInput")
    bv_d = nc.dram_tensor("bvr", [1, OC], F32, kind="ExternalInput")
    woR_d = nc.dram_tensor("woR", [OC, D], BF16, kind="ExternalInput")
    bo_d = nc.dram_tensor("bor", [1, D], F32, kind="ExternalInput")
    if masked:
        maskT_d = nc.dram_tensor("maskT", [S, S], BF16, kind="ExternalInput")
    out_d = nc.dram_tensor("out", [S, D], F32, kind="ExternalOutput")

    with tile.TileContext(nc) as tc:
        with (
            tc.tile_pool(name="persist", bufs=1) as persist,
            tc.tile_pool(name="wpool", bufs=1) as wpool,
            tc.tile_pool(name="xt", bufs=2) as xtp,
            tc.tile_pool(name="et", bufs=4) as etp,
            tc.tile_pool(name="rec", bufs=2) as recp,
            tc.tile_pool(name="outp", bufs=2) as outp,
            tc.tile_pool(name="mk", bufs=2) as mkp,
            tc.tile_pool(name="stps", bufs=2, space="PSUM") as stps,
            tc.tile_pool(name="avps", bufs=2, space="PSUM") as avps,
        ):
            # persistent SBUF tensors
            QT = [persist.tile([128, S], BF16, tag=f"QT{p}", name=f"QT{p}") for p in range(2)]
            KT = [persist.tile([128, S], BF16, tag=f"KT{p}", name=f"KT{p}") for p in range(2)]
            attnT = [persist.tile([128, S], BF16, tag=f"attnT{p}", name=f"attnT{p}") for p in range(2)]
            # per head-slot: cols 0:64 = V channels, cols 64:128 = ones
            Vsb = persist.tile([128, NKT, HG, 128], BF16, tag="Vsb")
            ones_row = persist.tile([1, 128], F32, tag="ones_row")
            bv_bc = persist.tile([128, OC], F32, tag="bv_bc")
            bo_bc = persist.tile([128, D], F32, tag="bo_bc")
            nc.gpsimd.memset(ones_row[:], 1.0)
            nc.gpsimd.memset(Vsb[:, :, :, 64:128], 1.0)

            wq_sb = wpool.tile([128, 8, OC], BF16, tag="wq")
            wk_sb = wpool.tile([128, 8, OC], BF16, tag="wk")
            wv_sb = wpool.tile([128, 8, OC], BF16, tag="wv")
            wo_sb = wpool.tile([128, 2, D], BF16, tag="wo")
            bq_sb = wpool.tile([128, 2], F32, tag="bq")
            bk_sb = wpool.tile([128, 2], F32, tag="bk")
            bv_sb = wpool.tile([1, OC], F32, tag="bv")
            bo_sb = wpool.tile([1, D], F32, tag="bo")
            nc.sync.dma_start(wq_sb[:], wqT_d.rearrange("(dc p) o -> p dc o", p=128))
            nc.sync.dma_start(wk_sb[:], wkT_d.rearrange("(dc p) o -> p dc o", p=128))
            nc.sync.dma_start(wv_sb[:], wvT_d.rearrange("(dc p) o -> p dc o", p=128))
            nc.sync.dma_start(wo_sb[:], woR_d.rearrange("(cc p) o -> p cc o", p=128))
            nc.sync.dma_start(bq_sb[:], bq_d[:])
            nc.sync.dma_start(bk_sb[:], bk_d[:])
            nc.sync.dma_start(bv_sb[:], bv_d[:])
            nc.sync.dma_start(bo_sb[:], bo_d[:])

            # broadcast bv / bo across partitions via K=1 matmuls
            ps = stps.tile([128, QBLK], F32, tag="st", name="st")
            nc.tensor.matmul(ps[:, 0:OC], ones_row[:], bv_sb[:], start=True, stop=True)
            nc.vector.tensor_copy(bv_bc[:], ps[:, 0:OC])
            ps = stps.tile([128, QBLK], F32, tag="st", name="st")
            for oh in range(2):
                nc.tensor.matmul(
                    ps[:, oh * 512 : (oh + 1) * 512],
                    ones_row[:],
                    bo_sb[:, oh * 512 : (oh + 1) * 512],
                    start=True,
                    stop=True,
                )
            nc.vector.tensor_copy(bo_bc[:], ps[:])

            # ---------------- Q/K projections ----------------
            for xd, wsb, bsb, dst in (
                (xqT_d, wq_sb, bq_sb, QT),
                (xkT_d, wk_sb, bk_sb, KT),
            ):
                xr = xd.rearrange("(dc p) t -> p dc t", p=128)
                for tt in range(8):  # 512-token tiles
                    xt = xtp.tile([128, 8, 512], BF16, tag="xqk", name="xqk")
                    nc.sync.dma_start(xt[:], xr[:, :, tt * 512 : (tt + 1) * 512])
                    for oc in range(2):
                        pst = stps.tile([128, QBLK], F32, tag="st", name="st")
                        for dc in range(8):
                            nc.tensor.matmul(
                                pst[:, 0:512],
                                wsb[:, dc, oc * 128 : (oc + 1) * 128],
                                xt[:, dc, :],
                                start=(dc == 0),
                                stop=(dc == 7),
                            )
                        nc.scalar.activation(
                            dst[oc][:, tt * 512 : (tt + 1) * 512],
                            pst[:, 0:512],
                            mybir.ActivationFunctionType.Identity,
                            bias=bsb[:, oc : oc + 1],
                        )

            # ---------------- V projection -> Vsb[:, kt, hs, 0:64] ----------
            xvr = xvT_d.rearrange("(dc p) t -> p dc t", p=128)
            for tcI in range(NKT):
                xt = xtp.tile([128, 8, 128], BF16, tag="xv", name="xv")
                nc.sync.dma_start(xt[:], xvr[:, :, tcI * 128 : (tcI + 1) * 128])
                pst = stps.tile([128, QBLK], F32, tag="st", name="st")
                for dc in range(8):
                    nc.tensor.matmul(
                        pst[:, 0:OC],
                        xt[:, dc, :],
                        wv_sb[:, dc, :],
                        start=(dc == 0),
                        stop=(dc == 7),
                    )
                for hs in range(HG):
                    nc.vector.tensor_add(
                        Vsb[:, tcI, hs, 0:64],
                        pst[:, hs * 64 : (hs + 1) * 64],
                        bv_bc[:, hs * 64 : (hs + 1) * 64],
                    )

            # ---------------- attention + output projection ----------------
            if masked:
                mrr = maskT_d.rearrange("(kt p) q -> kt p q", p=128)
            our = out_d.rearrange("(tt p) o -> tt p o", p=128)

            for qb in range(NQB):
                q0 = qb * QBLK
                for pair in range(2):
                    QTp, KTp, ATp = QT[pair], KT[pair], attnT[pair]
                    for h2 in range(2):
                        b0 = h2 * 64
                        hs = pair * 2 + h2
                        at = avps.tile([128, QBLK], F32, tag="at", name="at")
                        et_prev = None

                        def do_av(et, kt, start, stop):
                            for qh in range(2):
                                nc.tensor.matmul(
                                    at[:, qh * 512 : (qh + 1) * 512],
                                    Vsb[:, kt, hs, :],
                                    et[:, qh * 512 : (qh + 1) * 512],
                                    start=start,
                                    stop=stop,
                                )

                        for kt in range(NKT):
                            st = stps.tile([128, QBLK], F32, tag="st", name="st")
                            for qh in range(2):
                                nc.tensor.matmul(
                                    st[:, qh * 512 : (qh + 1) * 512],
                                    KTp[b0 : b0 + 64, kt * 128 : (kt + 1) * 128],
                                    QTp[b0 : b0 + 64, q0 + qh * 512 : q0 + (qh + 1) * 512],
                                    start=True,
                                    stop=True,
                                )
                            et = etp.tile([128, QBLK], BF16, tag="et", name="et")
                            nc.scalar.activation(
                                et[:],
                                st[:],
                                mybir.ActivationFunctionType.Exp,
                                scale=EXP_SCALE,
                            )
                            if masked:
                                mk = mkp.tile([128, QBLK], BF16, tag="mk", name="mk")
                                nc.sync.dma_start(mk[:], mrr[kt][:, q0 : q0 + QBLK])
                                nc.vector.tensor_mul(et[:], et[:], mk[:])
                            if kt > 0:
                                do_av(et_prev, kt - 1, start=(kt == 1), stop=False)
                            et_prev = et
                        do_av(et_prev, NKT - 1, start=False, stop=True)

                        # normalize: attn rows 0:64 / rowsum rows 64:128
                        rc = recp.tile([64, QBLK], F32, tag="rc", name="rc")
                        nc.vector.reciprocal(rc[:], at[64:128, :])
                        nc.vector.tensor_tensor(
                            ATp[b0 : b0 + 64, q0 : q0 + QBLK],
                            at[0:64, :],
                            rc[:],
                            mybir.AluOpType.mult,
                        )

                # output projection for this q-block's token tiles
                for tt in range(qb * 8, (qb + 1) * 8):
                    po = stps.tile([128, QBLK], F32, tag="st", name="st")
                    for oh in range(2):
                        for cc in range(2):
                            nc.tensor.matmul(
                                po[:, oh * 512 : (oh + 1) * 512],
                                attnT[cc][:, tt * 128 : (tt + 1) * 128],
                                wo_sb[:, cc, oh * 512 : (oh + 1) * 512],
                                start=(cc == 0),
                                stop=(cc == 1),
                            )
                    ot = outp.tile([128, D], F32, tag="ot", name="ot")
                    nc.vector.tensor_add(ot[:], po[:], bo_bc[:])
                    nc.sync.dma_start(our[tt], ot[:])

    return _split_waits(nc) if split_waits else nc


def _prep_in_maps(inputs):
    q = np.asarray(inputs["query"], np.float32)
    k = np.asarray(inputs["key"], np.float32)
    v = np.asarray(inputs["value"], np.float32)
    mask = np.asarray(inputs["mask"])
    Wq = np.asarray(inputs["Wq"], np.float32)
    Wk = np.asarray(inputs["Wk"], np.float32)
    Wv = np.asarray(inputs["Wv"], np.float32)
    Wo = np.asarray(inputs["Wo"], np.float32)
    bq = np.asarray(inputs["bq"], np.float32)
    bk = np.asarray(inputs["bk"], np.float32)
    bv = np.asarray(inputs["bv"], np.float32)
    bo = np.asarray(inputs["bo"], np.float32)

    masked = not bool((mask != 0).all())
    xT = {}
    for nm, x in (("q", q), ("k", k), ("v", v)):
        for b in range(B):
            xT[(nm, b)] = np.ascontiguousarray(x[b].T).astype(bf16)
    if masked:
        maskT = np.ascontiguousarray(
            (np.broadcast_to(mask[0, 0], (S, S)).T != 0)
        ).astype(bf16)

    in_maps = []
    for c in range(NCORES):
        b, hg = c // HG, c % HG
        sl = slice(hg * OC, (hg + 1) * OC)
        m = {
            "xqT": xT[("q", b)],
            "xkT": xT[("k", b)],
            "xvT": xT[("v", b)],
            "wqT": np.ascontiguousarray(Wq[sl].T).astype(bf16),
            "wkT": np.ascontiguousarray(Wk[sl].T).astype(bf16),
            "wvT": np.ascontiguousarray(Wv[sl].T).astype(bf16),
            "bq2": np.ascontiguousarray(bq[sl].reshape(2, 128).T),
            "bk2": np.ascontiguousarray(bk[sl].reshape(2, 128).T),
            "bvr": bv[sl].reshape(1, OC).copy(),
            "woR": np.ascontiguousarray(Wo[:, sl].T).astype(bf16),
            "bor": (bo if hg == 0 else np.zeros_like(bo)).reshape(1, D).copy(),
        }
        if masked:
            m["maskT"] = maskT
        in_maps.append(m)
    return in_maps, masked


def _install_profile_hook():
    """Provide antenv.axon_hooks + register the NTFF profile hook via ctypes
    against libaxon_pjrt.so (the agent image lacks antenv.axon_hooks, which
    makes run_bass_kernel_spmd(trace=True) fall over; see trn_boot.py)."""
    import types
    import ctypes
    import contextlib

    if "antenv.axon_hooks" in sys.modules:
        return
    mod = types.ModuleType("antenv.axon_hooks")
    state = {"hook": None}
    mod.set_axon_ntff_profile_hook = lambda h: state.__setitem__("hook", h)
    mod.get_axon_ntff_profile_hook = lambda: state["hook"]
    sys.modules["antenv.axon_hooks"] = mod

    so_path = "/opt/axon/libaxon_pjrt.so"
    if not os.path.exists(so_path):
        return
    lib = ctypes.CDLL(so_path)
    if not hasattr(lib, "axon_start_nrt_profile"):
        return
    lib.axon_start_nrt_profile.argtypes = [
        ctypes.POINTER(ctypes.c_int64),
        ctypes.c_size_t,
    ]
    lib.axon_start_nrt_profile.restype = ctypes.c_int64
    lib.axon_stop_nrt_profile.argtypes = [ctypes.c_char_p]
    lib.axon_stop_nrt_profile.restype = ctypes.c_int64

    @contextlib.contextmanager
    def _hook(output_dir, device_ids):
        import jax

        jax.devices()
        if device_ids:
            ids = (ctypes.c_int64 * len(device_ids))(*device_ids)
            rc = lib.axon_start_nrt_profile(ids, len(device_ids))
        else:
            rc = lib.axon_start_nrt_profile(None, 0)
        if rc != 0:
            raise RuntimeError(f"axon_start_nrt_profile rc={rc}")
        try:
            yield
        finally:
            n = lib.axon_stop_nrt_profile(str(output_dir).encode())
            print(f"profile: {n} file(s) written to {output_dir}", file=sys.stderr)

    mod.set_axon_ntff_profile_hook(_hook)


def run(inputs, trace=False):
    if trace:
        _install_profile_hook()
    in_maps, masked = _prep_in_maps(inputs)
    nc = _build(masked)
    res = bass_utils.run_bass_kernel_spmd(
        nc, in_maps, core_ids=list(range(NCORES)), trace=trace
    )
    out = np.zeros((B, S, D), np.float32)
    for c in range(NCORES):
        out[c // HG] += res.results[c]["out"]
    return out, res


def kernel(**inputs):
    return run(inputs, trace=False)[0]


# revision 8
# speedup vs baseline: 1.3620x; 1.3620x over previous
"""Trainium2 Bass kernel for nn_MultiHeadAttention (B=2, S=4096, D=1024, H=16, Dh=64).

Sharding over 8 cores: core c handles batch b=c//4 and head-group hg=c%4
(4 heads = 256 channels). Host gathers by summing the 4 per-head-group partial
output projections per batch (row-parallel output projection).

v3 design (dependency-stall optimized; microbench shows every matmul shape
sustains 216ns/512col back-to-back — prior slowness was semaphore stalls):
  - Heads processed sequentially per (q-block, pair); per kt-step the PE does
    [AV(kt-LAG), fillers, scores(kt)] so the only instruction that may wait
    (scores, on exp's PSUM rotation) sits after ready work.
  - AV lags exp by LAG=4 kt-steps (deep ET pool) so its dependency is long
    satisfied when the PE reaches it.
  - V||ones: AV matmuls also produce the softmax rowsum broadcast into PSUM
    rows 64..127 for free (matmul cost = output columns only).
  - Exp on ACT is the phase bottleneck (~1.1us per [128,1024] tile, 512
    tiles); ALL projection + output-projection tiles are JIT-scheduled as PE
    fillers inside the attention phase so the PE's surplus absorbs them.
  - Normalization: PSUM->SBUF stage copy (releases the single attn PSUM tile
    fast), then DVE reciprocal + multiply off the critical path.

PSUM: scores tag "st" 2x[128,1024] (4 banks) + filler tag "pj" 1x[128,1024]
(2 banks) + attn tag "at" 1x[128,1024] (2 banks) = 8 banks exactly.
"""

import math
import os
import sys
import functools

import numpy as np
import ml_dtypes

sys.path.insert(0, "/opt/trn_rl_repo")

import concourse.bass as bass  # noqa: E402
import concourse.mybir as mybir  # noqa: E402
import concourse.tile as tile  # noqa: E402
from concourse import bass_utils  # noqa: E402

B, S, D, H, DH = 2, 4096, 1024, 16, 64
NCORES = 8
HG = 4  # head groups (cores per batch)
OC = 256  # q/k/v channels per core
BF16 = mybir.dt.bfloat16
F32 = mybir.dt.float32
QBLK = 1024
NQB = S // QBLK  # 4
NKT = S // 128  # 32 k-tiles
bf16 = ml_dtypes.bfloat16
EXP_SCALE = 1.0 / math.sqrt(DH)
LAG = 4  # AV trails exp by this many kt-steps
NSEQ = NQB * 4  # 16 head-sequences


_TPB_ENGINES = None


def _split_waits(nc, max_waits=1):
    """walrus codegen in this container rejects TPB instructions carrying more
    than one sync-wait command.  Spill extra semaphore waits onto preceding
    NoOps on the same engine (engines execute their queue in order, so a NoOp
    that waits immediately before the instruction is equivalent)."""
    import bass_rust

    global _TPB_ENGINES
    if _TPB_ENGINES is None:
        _TPB_ENGINES = {
            mybir.EngineType.Pool,
            mybir.EngineType.Activation,
            mybir.EngineType.PE,
            mybir.EngineType.DVE,
            mybir.EngineType.SP,
        }
    ctr = 0
    for bb in nc.main_func.blocks:
        insts = bb.instructions
        out = []
        changed = False
        for inst in insts:
            si = getattr(inst, "sync_info", None)
            if (
                si is not None
                and si.on_wait
                and len(si.on_wait) > max_waits
                and inst.engine in _TPB_ENGINES
            ):
                waits = list(si.on_wait)
                keep = waits[-max_waits:]
                spill = waits[:-max_waits]
                for i in range(0, len(spill), max_waits):
                    nop = bass_rust.InstNoOp(
                        name=f"{inst.name}-sw{ctr}", ins=[], outs=[]
                    )
                    ctr += 1
                    nop.engine = inst.engine
                    nop.sync_info = mybir.SyncInfo(
                        on_wait=spill[i : i + max_waits], on_update=[]
                    )
                    out.append(nop)
                inst.sync_info = mybir.SyncInfo(
                    on_wait=keep, on_update=list(si.on_update)
                )
                changed = True
            out.append(inst)
        if changed:
            insts[:] = out
    return nc


@functools.lru_cache(maxsize=4)
def _build(masked: bool, split_waits: bool = True):
    nc = bass.Bass()

    xqT_d = nc.dram_tensor("xqT", [D, S], BF16, kind="ExternalInput")
    xkT_d = nc.dram_tensor("xkT", [D, S], BF16, kind="ExternalInput")
    xvT_d = nc.dram_tensor("xvT", [D, S], BF16, kind="ExternalInput")
    wqT_d = nc.dram_tensor("wqT", [D, OC], BF16, kind="ExternalInput")
    wkT_d = nc.dram_tensor("wkT", [D, OC], BF16, kind="ExternalInput")
    wvT_d = nc.dram_tensor("wvT", [D, OC], BF16, kind="ExternalInput")
    bq_d = nc.dram_tensor("bq2", [128, 2], F32, kind="ExternalInput")
    bk_d = nc.dram_tensor("bk2", [128, 2], F32, kind="ExternalInput")
    bv_d = nc.dram_tensor("bvr", [1, OC], F32, kind="ExternalInput")
    woR_d = nc.dram_tensor("woR", [OC, D], BF16, kind="ExternalInput")
    bo_d = nc.dram_tensor("bor", [1, D], F32, kind="ExternalInput")
    if masked:
        maskT_d = nc.dram_tensor("maskT", [S, S], BF16, kind="ExternalInput")
    out_d = nc.dram_tensor("out", [S, D], F32, kind="ExternalOutput")

    with tile.TileContext(nc) as tc:
        with (
            tc.tile_pool(name="persist", bufs=1) as persist,
            tc.tile_pool(name="wpool", bufs=1) as wpool,
            tc.tile_pool(name="xt", bufs=4) as xtp,
            tc.tile_pool(name="et", bufs=8) as etp,
            tc.tile_pool(name="rec", bufs=2) as recp,
            tc.tile_pool(name="stage", bufs=2) as stagep,
            tc.tile_pool(name="outp", bufs=2) as outp,
            tc.tile_pool(name="mk", bufs=2) as mkp,
            tc.tile_pool(name="psum", bufs=1, space="PSUM") as psp,
        ):
            # persistent SBUF tensors
            QT = [persist.tile([128, S], BF16, tag=f"QT{p}", name=f"QT{p}") for p in range(2)]
            KT = [persist.tile([128, S], BF16, tag=f"KT{p}", name=f"KT{p}") for p in range(2)]
            attnT = [persist.tile([128, S], BF16, tag=f"attnT{p}", name=f"attnT{p}") for p in range(2)]
            # per head-slot: cols 0:64 = V channels, cols 64:128 = ones
            Vsb = persist.tile([128, NKT, HG, 128], BF16, tag="Vsb")
            ones_row = persist.tile([1, 128], F32, tag="ones_row")
            bv_bc = persist.tile([128, OC], F32, tag="bv_bc")
            bo_bc = persist.tile([128, D], F32, tag="bo_bc")
            nc.gpsimd.memset(ones_row[:], 1.0)
            nc.gpsimd.memset(Vsb[:, :, :, 64:128], 1.0)

            wq_sb = wpool.tile([128, 8, OC], BF16, tag="wq")
            wk_sb = wpool.tile([128, 8, OC], BF16, tag="wk")
            wv_sb = wpool.tile([128, 8, OC], BF16, tag="wv")
            wo_sb = wpool.tile([128, 2, D], BF16, tag="wo")
            bq_sb = wpool.tile([128, 2], F32, tag="bq")
            bk_sb = wpool.tile([128, 2], F32, tag="bk")
            bv_sb = wpool.tile([1, OC], F32, tag="bv")
            bo_sb = wpool.tile([1, D], F32, tag="bo")
            nc.sync.dma_start(wq_sb[:], wqT_d.rearrange("(dc p) o -> p dc o", p=128))
            nc.sync.dma_start(wk_sb[:], wkT_d.rearrange("(dc p) o -> p dc o", p=128))
            nc.sync.dma_start(wv_sb[:], wvT_d.rearrange("(dc p) o -> p dc o", p=128))
            nc.sync.dma_start(wo_sb[:], woR_d.rearrange("(cc p) o -> p cc o", p=128))
            nc.sync.dma_start(bq_sb[:], bq_d[:])
            nc.sync.dma_start(bk_sb[:], bk_d[:])
            nc.sync.dma_start(bv_sb[:], bv_d[:])
            nc.sync.dma_start(bo_sb[:], bo_d[:])

            # broadcast bv / bo across partitions via K=1 matmuls
            ps = psp.tile([128, QBLK], F32, tag="pj", name="pj")
            nc.tensor.matmul(ps[:, 0:OC], ones_row[:], bv_sb[:], start=True, stop=True)
            nc.vector.tensor_copy(bv_bc[:], ps[:, 0:OC])
            ps = psp.tile([128, QBLK], F32, tag="pj", name="pj")
            for oh in range(2):
                nc.tensor.matmul(
                    ps[:, oh * 512 : (oh + 1) * 512],
                    ones_row[:],
                    bo_sb[:, oh * 512 : (oh + 1) * 512],
                    start=True,
                    stop=True,
                )
            nc.vector.tensor_copy(bo_bc[:], ps[:])

            # ---------------- filler units (projections + out-proj) --------
            xr = {
                "q": xqT_d.rearrange("(dc p) t -> p dc t", p=128),
                "k": xkT_d.rearrange("(dc p) t -> p dc t", p=128),
            }
            xvr = xvT_d.rearrange("(dc p) t -> p dc t", p=128)
            wmap = {"q": wq_sb, "k": wk_sb}
            bmap = {"q": bq_sb, "k": bk_sb}
            dstmap = {"q": QT, "k": KT}
            our = out_d.rearrange("(tt p) o -> tt p o", p=128)

            def qk_unit(which, oc, tt):
                holder = {}

                def dma_fn():
                    xt = xtp.tile([128, 8, 512], BF16, tag="xqk", name="xqk")
                    nc.sync.dma_start(
                        xt[:], xr[which][:, :, tt * 512 : (tt + 1) * 512]
                    )
                    holder["xt"] = xt

                def mm_fn():
                    xt = holder["xt"]
                    pst = psp.tile([128, QBLK], F32, tag="pj", name="pj")
                    for dc in range(8):
                        nc.tensor.matmul(
                            pst[:, 0:512],
                            wmap[which][:, dc, oc * 128 : (oc + 1) * 128],
                            xt[:, dc, :],
                            start=(dc == 0),
                            stop=(dc == 7),
                        )
                    nc.vector.tensor_scalar_add(
                        dstmap[which][oc][:, tt * 512 : (tt + 1) * 512],
                        pst[:, 0:512],
                        bmap[which][:, oc : oc + 1],
                    )

                return dma_fn, mm_fn

            def v_unit(tcI):
                holder = {}

                def dma_fn():
                    xt = xtp.tile([128, 8, 128], BF16, tag="xv", name="xv")
                    nc.sync.dma_start(
                        xt[:], xvr[:, :, tcI * 128 : (tcI + 1) * 128]
                    )
                    holder["xt"] = xt

                def mm_fn():
                    xt = holder["xt"]
                    pst = psp.tile([128, QBLK], F32, tag="pj", name="pj")
                    for dc in range(8):
                        nc.tensor.matmul(
                            pst[:, 0:OC],
                            xt[:, dc, :],
                            wv_sb[:, dc, :],
                            start=(dc == 0),
                            stop=(dc == 7),
                        )
                    # one add into the 4-way strided V slots (free sizes match)
                    nc.vector.tensor_add(
                        Vsb[:, tcI, :, 0:64], pst[:, 0:OC], bv_bc[:]
                    )

                return dma_fn, mm_fn

            def op_unit(tt):
                def mm_fn():
                    po = psp.tile([128, QBLK], F32, tag="pj", name="pj")
                    for oh in range(2):
                        for cc in range(2):
                            nc.tensor.matmul(
                                po[:, oh * 512 : (oh + 1) * 512],
                                attnT[cc][:, tt * 128 : (tt + 1) * 128],
                                wo_sb[:, cc, oh * 512 : (oh + 1) * 512],
                                start=(cc == 0),
                                stop=(cc == 1),
                            )
                    ot = outp.tile([128, D], F32, tag="ot", name="ot")
                    nc.vector.tensor_add(ot[:], po[:], bo_bc[:])
                    nc.sync.dma_start(our[tt], ot[:])

                return None, mm_fn

            # ---------------- schedule -------------------------------------
            # sched[seq][step] = list of closures to emit after AV, before
            # scores, at that kt-step.
            sched = [[[] for _ in range(NKT)] for _ in range(NSEQ)]
            prologue = []

            def place(si, step, fn):
                if fn is None:
                    return
                if si < 0 or (si == 0 and step < 0):
                    prologue.append(fn)
                elif step < 0:
                    sched[si - 1][NKT + step].append(fn)
                elif step >= NKT:
                    sched[si + 1][step - NKT].append(fn)
                else:
                    sched[si][step].append(fn)

            def place_unit(si, step, unit, lead=2):
                dma_fn, mm_fn = unit
                if dma_fn is not None:
                    place(si, step - lead, dma_fn)
                place(si, step, mm_fn)

            # prologue units: K(0,0), Q(0,0), Q(0,1), V(0), V(1)
            pro_units = [qk_unit("k", 0, 0), qk_unit("q", 0, 0),
                         qk_unit("q", 0, 1), v_unit(0), v_unit(1)]
            for u in pro_units:
                if u[0] is not None:
                    prologue.append(u[0])
            for u in pro_units:
                prologue.append(u[1])

            # seq0: V tc2..31 at step tc-2; K(0,tt) tt1..7 at 4(tt-1);
            #       Q(1,0)@2, Q(1,1)@6
            for tcI in range(2, NKT):
                place_unit(0, tcI - 2, v_unit(tcI))
            for tt in range(1, 8):
                place_unit(0, 4 * (tt - 1) + 1, qk_unit("k", 0, tt))
            place_unit(0, 2, qk_unit("q", 1, 0))
            place_unit(0, 6, qk_unit("q", 1, 1))
            # seq1: K(1,tt) tt0..7 at 4tt
            for tt in range(8):
                place_unit(1, 4 * tt, qk_unit("k", 1, tt))
            # seq2/3: Q tiles for qb1
            place_unit(2, 0, qk_unit("q", 0, 2))
            place_unit(2, 8, qk_unit("q", 1, 2))
            place_unit(2, 16, qk_unit("q", 0, 3))
            place_unit(2, 24, qk_unit("q", 1, 3))
            # Q tiles for qb2 in seqs 4-5, qb3 in seqs 6-7
            place_unit(4, 6, qk_unit("q", 0, 4))
            place_unit(4, 22, qk_unit("q", 1, 4))
            place_unit(5, 6, qk_unit("q", 0, 5))
            place_unit(5, 22, qk_unit("q", 1, 5))
            place_unit(6, 6, qk_unit("q", 0, 6))
            place_unit(6, 22, qk_unit("q", 1, 6))
            place_unit(7, 6, qk_unit("q", 0, 7))
            place_unit(7, 22, qk_unit("q", 1, 7))
            # out-proj: qb emitted during seqs 4(qb+1)..+1
            for qb in range(NQB - 1):
                for j in range(8):
                    si = 4 * (qb + 1) + j // 4
                    place_unit(si, 2 + 8 * (j % 4), op_unit(qb * 8 + j))

            for fn in prologue:
                fn()

            # ---------------- attention ------------------------------------
            if masked:
                mrr = maskT_d.rearrange("(kt p) q -> kt p q", p=128)

            for qb in range(NQB):
                q0 = qb * QBLK
                for pair in range(2):
                    QTp, KTp, ATp = QT[pair], KT[pair], attnT[pair]
                    for h2 in range(2):
                        si = (qb * 2 + pair) * 2 + h2
                        b0 = h2 * 64
                        hs = pair * 2 + h2
                        at = psp.tile([128, QBLK], F32, tag="at", name="at")
                        ets = {}

                        def do_av(kt, start, stop):
                            et = ets.pop(kt)
                            for qh in range(2):
                                nc.tensor.matmul(
                                    at[:, qh * 512 : (qh + 1) * 512],
                                    Vsb[:, kt, hs, :],
                                    et[:, qh * 512 : (qh + 1) * 512],
                                    start=start,
                                    stop=stop,
                                )

                        for kt in range(NKT):
                            if kt >= LAG:
                                do_av(kt - LAG, start=(kt == LAG), stop=False)
                            for fn in sched[si][kt]:
                                fn()
                            st = psp.tile([128, QBLK], F32, tag="st", bufs=2, name="st")
                            for qh in range(2):
                                nc.tensor.matmul(
                                    st[:, qh * 512 : (qh + 1) * 512],
                                    KTp[b0 : b0 + 64, kt * 128 : (kt + 1) * 128],
                                    QTp[b0 : b0 + 64, q0 + qh * 512 : q0 + (qh + 1) * 512],
                                    start=True,
                                    stop=True,
                                )
                            et = etp.tile([128, QBLK], BF16, tag="et", name="et")
                            nc.scalar.activation(
                                et[:],
                                st[:],
                                mybir.ActivationFunctionType.Exp,
                                scale=EXP_SCALE,
                            )
                            if masked:
                                mk = mkp.tile([128, QBLK], BF16, tag="mk", name="mk")
                                nc.sync.dma_start(mk[:], mrr[kt][:, q0 : q0 + QBLK])
                                nc.vector.tensor_mul(et[:], et[:], mk[:])
                            ets[kt] = et
                        for kt in range(NKT - LAG, NKT):
                            do_av(kt, start=False, stop=(kt == NKT - 1))

                        # release the attn PSUM tile fast, normalize off-path
                        stg = stagep.tile([128, QBLK], F32, tag="stage", name="stage")
                        nc.vector.tensor_copy(stg[:], at[:])
                        rc = recp.tile([64, QBLK], F32, tag="rc", name="rc")
                        nc.vector.reciprocal(rc[:], stg[64:128, :])
                        nc.vector.tensor_tensor(
                            ATp[b0 : b0 + 64, q0 : q0 + QBLK],
                            stg[0:64, :],
                            rc[:],
                            mybir.AluOpType.mult,
                        )

            # tail: out-proj for qb3
            for tt in range(24, 32):
                op_unit(tt)[1]()

    return _split_waits(nc) if split_waits else nc


def _prep_in_maps(inputs):
    q = np.asarray(inputs["query"], np.float32)
    k = np.asarray(inputs["key"], np.float32)
    v = np.asarray(inputs["value"], np.float32)
    mask = np.asarray(inputs["mask"])
    Wq = np.asarray(inputs["Wq"], np.float32)
    Wk = np.asarray(inputs["Wk"], np.float32)
    Wv = np.asarray(inputs["Wv"], np.float32)
    Wo = np.asarray(inputs["Wo"], np.float32)
    bq = np.asarray(inputs["bq"], np.float32)
    bk = np.asarray(inputs["bk"], np.float32)
    bv = np.asarray(inputs["bv"], np.float32)
    bo = np.asarray(inputs["bo"], np.float32)

    masked = not bool((mask != 0).all())
    xT = {}
    for nm, x in (("q", q), ("k", k), ("v", v)):
        for b in range(B):
            xT[(nm, b)] = np.ascontiguousarray(x[b].T).astype(bf16)
    if masked:
        maskT = np.ascontiguousarray(
            (np.broadcast_to(mask[0, 0], (S, S)).T != 0)
        ).astype(bf16)

    in_maps = []
    for c in range(NCORES):
        b, hg = c // HG, c % HG
        sl = slice(hg * OC, (hg + 1) * OC)
        m = {
            "xqT": xT[("q", b)],
            "xkT": xT[("k", b)],
            "xvT": xT[("v", b)],
            "wqT": np.ascontiguousarray(Wq[sl].T).astype(bf16),
            "wkT": np.ascontiguousarray(Wk[sl].T).astype(bf16),
            "wvT": np.ascontiguousarray(Wv[sl].T).astype(bf16),
            "bq2": np.ascontiguousarray(bq[sl].reshape(2, 128).T),
            "bk2": np.ascontiguousarray(bk[sl].reshape(2, 128).T),
            "bvr": bv[sl].reshape(1, OC).copy(),
            "woR": np.ascontiguousarray(Wo[:, sl].T).astype(bf16),
            "bor": (bo if hg == 0 else np.zeros_like(bo)).reshape(1, D).copy(),
        }
        if masked:
            m["maskT"] = maskT
        in_maps.append(m)
    return in_maps, masked


def _install_profile_hook():
    """Provide antenv.axon_hooks + register the NTFF profile hook via ctypes
    against libaxon_pjrt.so (the agent image lacks antenv.axon_hooks, which
    makes run_bass_kernel_spmd(trace=True) fall over; see trn_boot.py)."""
    import types
    import ctypes
    import contextlib

    if "antenv.axon_hooks" in sys.modules:
        return
    mod = types.ModuleType("antenv.axon_hooks")
    state = {"hook": None}
    mod.set_axon_ntff_profile_hook = lambda h: state.__setitem__("hook", h)
    mod.get_axon_ntff_profile_hook = lambda: state["hook"]
    sys.modules["antenv.axon_hooks"] = mod

    so_path = "/opt/axon/libaxon_pjrt.so"
    if not os.path.exists(so_path):
        return
    lib = ctypes.CDLL(so_path)
    if not hasattr(lib, "axon_start_nrt_profile"):
        return
    lib.axon_start_nrt_profile.argtypes = [
        ctypes.POINTER(ctypes.c_int64),
        ctypes.c_size_t,
    ]
    lib.axon_start_nrt_profile.restype = ctypes.c_int64
    lib.axon_stop_nrt_profile.argtypes = [ctypes.c_char_p]
    lib.axon_stop_nrt_profile.restype = ctypes.c_int64

    @contextlib.contextmanager
    def _hook(output_dir, device_ids):
        import jax

        jax.devices()
        if device_ids:
            ids = (ctypes.c_int64 * len(device_ids))(*device_ids)
            rc = lib.axon_start_nrt_profile(ids, len(device_ids))
        else:
            rc = lib.axon_start_nrt_profile(None, 0)
        if rc != 0:
            raise RuntimeError(f"axon_start_nrt_profile rc={rc}")
        try:
            yield
        finally:
            n = lib.axon_stop_nrt_profile(str(output_dir).encode())
            print(f"profile: {n} file(s) written to {output_dir}", file=sys.stderr)

    mod.set_axon_ntff_profile_hook(_hook)


def run(inputs, trace=False):
    if trace:
        _install_profile_hook()
    in_maps, masked = _prep_in_maps(inputs)
    nc = _build(masked)
    res = bass_utils.run_bass_kernel_spmd(
        nc, in_maps, core_ids=list(range(NCORES)), trace=trace
    )
    out = np.zeros((B, S, D), np.float32)
    for c in range(NCORES):
        out[c // HG] += res.results[c]["out"]
    return out, res


def kernel(**inputs):
    return run(inputs, trace=False)[0]


# revision 14
# speedup vs baseline: 1.4205x; 1.0430x over previous
"""Trainium2 Bass kernel for nn_MultiHeadAttention (B=2, S=4096, D=1024, H=16, Dh=64).

Sharding over 8 cores: core c handles batch b=c//4 and head-group hg=c%4
(4 heads = 256 channels). Host gathers by summing the 4 per-head-group partial
output projections per batch (row-parallel output projection).

v3 design (dependency-stall optimized; microbench shows every matmul shape
sustains 216ns/512col back-to-back — prior slowness was semaphore stalls):
  - Heads processed sequentially per (q-block, pair); per kt-step the PE does
    [AV(kt-LAG), fillers, scores(kt)] so the only instruction that may wait
    (scores, on exp's PSUM rotation) sits after ready work.
  - AV lags exp by LAG=4 kt-steps (deep ET pool) so its dependency is long
    satisfied when the PE reaches it.
  - V||ones: AV matmuls also produce the softmax rowsum broadcast into PSUM
    rows 64..127 for free (matmul cost = output columns only).
  - Exp on ACT is the phase bottleneck (~1.1us per [128,1024] tile, 512
    tiles); ALL projection + output-projection tiles are JIT-scheduled as PE
    fillers inside the attention phase so the PE's surplus absorbs them.
  - Normalization: PSUM->SBUF stage copy (releases the single attn PSUM tile
    fast), then DVE reciprocal + multiply off the critical path.

PSUM: scores tag "st" 2x[128,1024] (4 banks) + filler tag "pj" 1x[128,1024]
(2 banks) + attn tag "at" 1x[128,1024] (2 banks) = 8 banks exactly.
"""

import math
import os
import sys
import functools

import numpy as np
import ml_dtypes

sys.path.insert(0, "/opt/trn_rl_repo")

import concourse.bass as bass  # noqa: E402
import concourse.mybir as mybir  # noqa: E402
import concourse.tile as tile  # noqa: E402
from concourse import bass_utils  # noqa: E402

B, S, D, H, DH = 2, 4096, 1024, 16, 64
NCORES = 8
HG = 4  # head groups (cores per batch)
OC = 256  # q/k/v channels per core
BF16 = mybir.dt.bfloat16
F32 = mybir.dt.float32
QBLK = 1024
NQB = S // QBLK  # 4
NKT = S // 128  # 32 k-tiles
bf16 = ml_dtypes.bfloat16
EXP_SCALE = 1.0 / math.sqrt(DH)
LAG = 4  # AV trails exp by this many kt-steps
NSEQ = NQB * 4  # 16 head-sequences


_TPB_ENGINES = None


def _split_waits(nc, max_waits=1):
    """walrus codegen in this container rejects TPB instructions carrying more
    than one sync-wait command.  Spill extra semaphore waits onto preceding
    NoOps on the same engine (engines execute their queue in order, so a NoOp
    that waits immediately before the instruction is equivalent)."""
    import bass_rust

    global _TPB_ENGINES
    if _TPB_ENGINES is None:
        _TPB_ENGINES = {
            mybir.EngineType.Pool,
            mybir.EngineType.Activation,
            mybir.EngineType.PE,
            mybir.EngineType.DVE,
            mybir.EngineType.SP,
        }
    ctr = 0
    for bb in nc.main_func.blocks:
        insts = bb.instructions
        out = []
        changed = False
        for inst in insts:
            si = getattr(inst, "sync_info", None)
            if (
                si is not None
                and si.on_wait
                and len(si.on_wait) > max_waits
                and inst.engine in _TPB_ENGINES
            ):
                waits = list(si.on_wait)
                keep = waits[-max_waits:]
                spill = waits[:-max_waits]
                for i in range(0, len(spill), max_waits):
                    nop = bass_rust.InstNoOp(
                        name=f"{inst.name}-sw{ctr}", ins=[], outs=[]
                    )
                    ctr += 1
                    nop.engine = inst.engine
                    nop.sync_info = mybir.SyncInfo(
                        on_wait=spill[i : i + max_waits], on_update=[]
                    )
                    out.append(nop)
                inst.sync_info = mybir.SyncInfo(
                    on_wait=keep, on_update=list(si.on_update)
                )
                changed = True
            out.append(inst)
        if changed:
            insts[:] = out
    return nc


@functools.lru_cache(maxsize=4)
def _build(masked: bool, split_waits: bool = True):
    nc = bass.Bass()

    xqT_d = nc.dram_tensor("xqT", [D, S], BF16, kind="ExternalInput")
    xkT_d = nc.dram_tensor("xkT", [D, S], BF16, kind="ExternalInput")
    xvT_d = nc.dram_tensor("xvT", [D, S], BF16, kind="ExternalInput")
    wqT_d = nc.dram_tensor("wqT", [D, OC], BF16, kind="ExternalInput")
    wkT_d = nc.dram_tensor("wkT", [D, OC], BF16, kind="ExternalInput")
    wvT_d = nc.dram_tensor("wvT", [D, OC], BF16, kind="ExternalInput")
    bq_d = nc.dram_tensor("bq2", [128, 2], F32, kind="ExternalInput")
    bk_d = nc.dram_tensor("bk2", [128, 2], F32, kind="ExternalInput")
    bv_d = nc.dram_tensor("bvr", [1, OC], F32, kind="ExternalInput")
    woR_d = nc.dram_tensor("woR", [OC, D], BF16, kind="ExternalInput")
    bo_d = nc.dram_tensor("bor", [1, D], F32, kind="ExternalInput")
    if masked:
        maskT_d = nc.dram_tensor("maskT", [S, S], BF16, kind="ExternalInput")
    out_d = nc.dram_tensor("out", [S, D], F32, kind="ExternalOutput")

    with tile.TileContext(nc) as tc:
        with (
            tc.tile_pool(name="persist", bufs=1) as persist,
            tc.tile_pool(name="wpool", bufs=1) as wpool,
            tc.tile_pool(name="xt", bufs=4) as xtp,
            tc.tile_pool(name="et", bufs=8) as etp,
            tc.tile_pool(name="rec", bufs=2) as recp,
            tc.tile_pool(name="stage", bufs=2) as stagep,
            tc.tile_pool(name="outp", bufs=2) as outp,
            tc.tile_pool(name="mk", bufs=2) as mkp,
            tc.tile_pool(name="psum", bufs=1, space="PSUM") as psp,
        ):
            # persistent SBUF tensors
            QT = [persist.tile([128, S], BF16, tag=f"QT{p}", name=f"QT{p}") for p in range(2)]
            KT = [persist.tile([128, S], BF16, tag=f"KT{p}", name=f"KT{p}") for p in range(2)]
            attnT = [persist.tile([128, S], BF16, tag=f"attnT{p}", name=f"attnT{p}") for p in range(2)]
            # per head-slot: cols 0:64 = V channels, cols 64:128 = ones
            Vsb = persist.tile([128, NKT, HG, 128], BF16, tag="Vsb")
            ones_row = persist.tile([1, 128], F32, tag="ones_row")
            bv_bc = persist.tile([128, OC], F32, tag="bv_bc")
            bo_bc = persist.tile([128, D], F32, tag="bo_bc")
            nc.gpsimd.memset(ones_row[:], 1.0)
            nc.gpsimd.memset(Vsb[:, :, :, 64:128], 1.0)

            wq_sb = wpool.tile([128, 8, OC], BF16, tag="wq")
            wk_sb = wpool.tile([128, 8, OC], BF16, tag="wk")
            wv_sb = wpool.tile([128, 8, OC], BF16, tag="wv")
            wo_sb = wpool.tile([128, 2, D], BF16, tag="wo")
            bq_sb = wpool.tile([128, 2], F32, tag="bq")
            bk_sb = wpool.tile([128, 2], F32, tag="bk")
            bv_sb = wpool.tile([1, OC], F32, tag="bv")
            bo_sb = wpool.tile([1, D], F32, tag="bo")
            nc.sync.dma_start(wq_sb[:], wqT_d.rearrange("(dc p) o -> p dc o", p=128))
            nc.sync.dma_start(wk_sb[:], wkT_d.rearrange("(dc p) o -> p dc o", p=128))
            nc.sync.dma_start(wv_sb[:], wvT_d.rearrange("(dc p) o -> p dc o", p=128))
            nc.sync.dma_start(wo_sb[:], woR_d.rearrange("(cc p) o -> p cc o", p=128))
            nc.sync.dma_start(bq_sb[:], bq_d[:])
            nc.sync.dma_start(bk_sb[:], bk_d[:])
            nc.sync.dma_start(bv_sb[:], bv_d[:])
            nc.sync.dma_start(bo_sb[:], bo_d[:])

            # broadcast bv / bo across partitions via K=1 matmuls
            ps = psp.tile([128, QBLK], F32, tag="pj", name="pj")
            nc.tensor.matmul(ps[:, 0:OC], ones_row[:], bv_sb[:], start=True, stop=True)
            nc.vector.tensor_copy(bv_bc[:], ps[:, 0:OC])
            ps = psp.tile([128, QBLK], F32, tag="pj", name="pj")
            for oh in range(2):
                nc.tensor.matmul(
                    ps[:, oh * 512 : (oh + 1) * 512],
                    ones_row[:],
                    bo_sb[:, oh * 512 : (oh + 1) * 512],
                    start=True,
                    stop=True,
                )
            nc.vector.tensor_copy(bo_bc[:], ps[:])

            # ---------------- filler units (projections + out-proj) --------
            xr = {
                "q": xqT_d.rearrange("(dc p) t -> p dc t", p=128),
                "k": xkT_d.rearrange("(dc p) t -> p dc t", p=128),
            }
            xvr = xvT_d.rearrange("(dc p) t -> p dc t", p=128)
            wmap = {"q": wq_sb, "k": wk_sb}
            bmap = {"q": bq_sb, "k": bk_sb}
            dstmap = {"q": QT, "k": KT}
            our = out_d.rearrange("(tt p) o -> tt p o", p=128)

            def qk_unit(which, oc, tt, holder=None, own_dma=True):
                if holder is None:
                    holder = {}

                def dma_fn():
                    xt = xtp.tile([128, 8, 512], BF16, tag="xqk", name="xqk")
                    nc.sync.dma_start(
                        xt[:], xr[which][:, :, tt * 512 : (tt + 1) * 512]
                    )
                    holder["xt"] = xt

                def mm_fn():
                    xt = holder["xt"]
                    pst = psp.tile([128, QBLK], F32, tag="pj", name="pj")
                    for dc in range(8):
                        nc.tensor.matmul(
                            pst[:, 0:512],
                            wmap[which][:, dc, oc * 128 : (oc + 1) * 128],
                            xt[:, dc, :],
                            start=(dc == 0),
                            stop=(dc == 7),
                        )
                    nc.vector.tensor_scalar_add(
                        dstmap[which][oc][:, tt * 512 : (tt + 1) * 512],
                        pst[:, 0:512],
                        bmap[which][:, oc : oc + 1],
                    )

                return (dma_fn if own_dma else None), mm_fn

            def v_unit(tcI):
                holder = {}

                def dma_fn():
                    xt = xtp.tile([128, 8, 128], BF16, tag="xv", name="xv")
                    nc.sync.dma_start(
                        xt[:], xvr[:, :, tcI * 128 : (tcI + 1) * 128]
                    )
                    holder["xt"] = xt

                def mm_fn():
                    xt = holder["xt"]
                    pst = psp.tile([128, QBLK], F32, tag="pj", name="pj")
                    for dc in range(8):
                        nc.tensor.matmul(
                            pst[:, 0:OC],
                            xt[:, dc, :],
                            wv_sb[:, dc, :],
                            start=(dc == 0),
                            stop=(dc == 7),
                        )
                    # one add into the 4-way strided V slots (free sizes match)
                    nc.vector.tensor_add(
                        Vsb[:, tcI, :, 0:64], pst[:, 0:OC], bv_bc[:]
                    )

                return dma_fn, mm_fn

            def op_unit(tt):
                def mm_fn():
                    po = psp.tile([128, QBLK], F32, tag="pj", name="pj")
                    for oh in range(2):
                        for cc in range(2):
                            nc.tensor.matmul(
                                po[:, oh * 512 : (oh + 1) * 512],
                                attnT[cc][:, tt * 128 : (tt + 1) * 128],
                                wo_sb[:, cc, oh * 512 : (oh + 1) * 512],
                                start=(cc == 0),
                                stop=(cc == 1),
                            )
                    ot = outp.tile([128, D], F32, tag="ot", name="ot")
                    nc.vector.tensor_add(ot[:], po[:], bo_bc[:])
                    nc.sync.dma_start(our[tt], ot[:])

                return None, mm_fn

            # ---------------- schedule -------------------------------------
            # sched[seq][step] = list of closures to emit after AV, before
            # scores, at that kt-step.
            sched = [[[] for _ in range(NKT)] for _ in range(NSEQ)]
            prologue = []

            def place(si, step, fn):
                if fn is None:
                    return
                if si < 0 or (si == 0 and step < 0):
                    prologue.append(fn)
                elif step < 0:
                    sched[si - 1][NKT + step].append(fn)
                elif step >= NKT:
                    sched[si + 1][step - NKT].append(fn)
                else:
                    sched[si][step].append(fn)

            def place_unit(si, step, unit, lead=4):
                dma_fn, mm_fn = unit
                if dma_fn is not None:
                    place(si, step - lead, dma_fn)
                place(si, step, mm_fn)

            # prologue units: K(0,0), Q(0,0), Q(0,1), V(0), V(1)
            pro_units = [qk_unit("k", 0, 0), qk_unit("q", 0, 0),
                         qk_unit("q", 0, 1), v_unit(0), v_unit(1)]
            for u in pro_units:
                if u[0] is not None:
                    prologue.append(u[0])
            for u in pro_units:
                prologue.append(u[1])

            # seq0: V tc2..31 at step tc-2; K(0,tt) tt1..7 at 4(tt-1);
            #       Q(1,0)@2, Q(1,1)@6
            for tcI in range(2, NKT):
                place_unit(0, tcI - 2, v_unit(tcI))
            for tt in range(1, 8):
                place_unit(0, 4 * (tt - 1) + 1, qk_unit("k", 0, tt))
            place_unit(0, 2, qk_unit("q", 1, 0))
            place_unit(0, 6, qk_unit("q", 1, 1))
            # seq1: K(1,tt) tt0..7 at 4tt
            for tt in range(8):
                place_unit(1, 4 * tt, qk_unit("k", 1, tt))
            # Q tiles tt2..7: both oc units share one DMA, mms 2 steps apart
            def place_q_pair(si, step, tt):
                holder = {}
                d0, m0 = qk_unit("q", 0, tt, holder=holder)
                _, m1 = qk_unit("q", 1, tt, holder=holder, own_dma=False)
                place(si, step - 4, d0)
                place(si, step, m0)
                place(si, step + 2, m1)

            place_q_pair(2, 4, 2)
            place_q_pair(2, 20, 3)
            place_q_pair(4, 4, 4)
            place_q_pair(5, 4, 5)
            place_q_pair(6, 4, 6)
            place_q_pair(7, 4, 7)
            # out-proj: qb emitted during seqs 4(qb+1)..+1, late enough that
            # the last normalize (copy+recip+mult ~9us) has drained
            for qb in range(NQB - 1):
                for j in range(8):
                    si = 4 * (qb + 1) + j // 4
                    place_unit(si, 12 + 5 * (j % 4), op_unit(qb * 8 + j))

            for fn in prologue:
                fn()

            # ---------------- attention ------------------------------------
            if masked:
                mrr = maskT_d.rearrange("(kt p) q -> kt p q", p=128)

            for qb in range(NQB):
                q0 = qb * QBLK
                for pair in range(2):
                    QTp, KTp, ATp = QT[pair], KT[pair], attnT[pair]
                    for h2 in range(2):
                        si = (qb * 2 + pair) * 2 + h2
                        b0 = h2 * 64
                        hs = pair * 2 + h2
                        at = psp.tile([128, QBLK], F32, tag="at", name="at")
                        ets = {}

                        def do_av(kt, start, stop):
                            et = ets.pop(kt)
                            for qh in range(2):
                                nc.tensor.matmul(
                                    at[:, qh * 512 : (qh + 1) * 512],
                                    Vsb[:, kt, hs, :],
                                    et[:, qh * 512 : (qh + 1) * 512],
                                    start=start,
                                    stop=stop,
                                )

                        for kt in range(NKT):
                            if kt >= LAG:
                                do_av(kt - LAG, start=(kt == LAG), stop=False)
                            st = psp.tile([128, QBLK], F32, tag="st", bufs=2, name="st")
                            for qh in range(2):
                                nc.tensor.matmul(
                                    st[:, qh * 512 : (qh + 1) * 512],
                                    KTp[b0 : b0 + 64, kt * 128 : (kt + 1) * 128],
                                    QTp[b0 : b0 + 64, q0 + qh * 512 : q0 + (qh + 1) * 512],
                                    start=True,
                                    stop=True,
                                )
                            et = etp.tile([128, QBLK], BF16, tag="et", name="et")
                            nc.scalar.activation(
                                et[:],
                                st[:],
                                mybir.ActivationFunctionType.Exp,
                                scale=EXP_SCALE,
                            )
                            if masked:
                                mk = mkp.tile([128, QBLK], BF16, tag="mk", name="mk")
                                nc.sync.dma_start(mk[:], mrr[kt][:, q0 : q0 + QBLK])
                                nc.vector.tensor_mul(et[:], et[:], mk[:])
                            ets[kt] = et
                            for fn in sched[si][kt]:
                                fn()
                        for kt in range(NKT - LAG, NKT):
                            do_av(kt, start=False, stop=(kt == NKT - 1))

                        # release the attn PSUM tile fast, normalize off-path
                        stg = stagep.tile([128, QBLK], F32, tag="stage", name="stage")
                        nc.vector.tensor_copy(stg[:], at[:])
                        rc = recp.tile([64, QBLK], F32, tag="rc", name="rc")
                        nc.vector.reciprocal(rc[:], stg[64:128, :])
                        nc.vector.tensor_tensor(
                            ATp[b0 : b0 + 64, q0 : q0 + QBLK],
                            stg[0:64, :],
                            rc[:],
                            mybir.AluOpType.mult,
                        )

            # tail: out-proj for qb3
            for tt in range(24, 32):
                op_unit(tt)[1]()

    return _split_waits(nc) if split_waits else nc


def _prep_in_maps(inputs):
    q = np.asarray(inputs["query"], np.float32)
    k = np.asarray(inputs["key"], np.float32)
    v = np.asarray(inputs["value"], np.float32)
    mask = np.asarray(inputs["mask"])
    Wq = np.asarray(inputs["Wq"], np.float32)
    Wk = np.asarray(inputs["Wk"], np.float32)
    Wv = np.asarray(inputs["Wv"], np.float32)
    Wo = np.asarray(inputs["Wo"], np.float32)
    bq = np.asarray(inputs["bq"], np.float32)
    bk = np.asarray(inputs["bk"], np.float32)
    bv = np.asarray(inputs["bv"], np.float32)
    bo = np.asarray(inputs["bo"], np.float32)

    masked = not bool((mask != 0).all())
    xT = {}
    for nm, x in (("q", q), ("k", k), ("v", v)):
        for b in range(B):
            xT[(nm, b)] = np.ascontiguousarray(x[b].T).astype(bf16)
    if masked:
        maskT = np.ascontiguousarray(
            (np.broadcast_to(mask[0, 0], (S, S)).T != 0)
        ).astype(bf16)

    in_maps = []
    for c in range(NCORES):
        b, hg = c // HG, c % HG
        sl = slice(hg * OC, (hg + 1) * OC)
        m = {
            "xqT": xT[("q", b)],
            "xkT": xT[("k", b)],
            "xvT": xT[("v", b)],
            "wqT": np.ascontiguousarray(Wq[sl].T).astype(bf16),
            "wkT": np.ascontiguousarray(Wk[sl].T).astype(bf16),
            "wvT": np.ascontiguousarray(Wv[sl].T).astype(bf16),
            "bq2": np.ascontiguousarray(bq[sl].reshape(2, 128).T),
            "bk2": np.ascontiguousarray(bk[sl].reshape(2, 128).T),
            "bvr": bv[sl].reshape(1, OC).copy(),
            "woR": np.ascontiguousarray(Wo[:, sl].T).astype(bf16),
            "bor": (bo if hg == 0 else np.zeros_like(bo)).reshape(1, D).copy(),
        }
        if masked:
            m["maskT"] = maskT
        in_maps.append(m)
    return in_maps, masked


def _install_profile_hook():
    """Provide antenv.axon_hooks + register the NTFF profile hook via ctypes
    against libaxon_pjrt.so (the agent image lacks antenv.axon_hooks, which
    makes run_bass_kernel_spmd(trace=True) fall over; see trn_boot.py)."""
    import types
    import ctypes
    import contextlib

    if "antenv.axon_hooks" in sys.modules:
        return
    mod = types.ModuleType("antenv.axon_hooks")
    state = {"hook": None}
    mod.set_axon_ntff_profile_hook = lambda h: state.__setitem__("hook", h)
    mod.get_axon_ntff_profile_hook = lambda: state["hook"]
    sys.modules["antenv.axon_hooks"] = mod

    so_path = "/opt/axon/libaxon_pjrt.so"
    if not os.path.exists(so_path):
        return
    lib = ctypes.CDLL(so_path)
    if not hasattr(lib, "axon_start_nrt_profile"):
        return
    lib.axon_start_nrt_profile.argtypes = [
        ctypes.POINTER(ctypes.c_int64),
        ctypes.c_size_t,
    ]
    lib.axon_start_nrt_profile.restype = ctypes.c_int64
    lib.axon_stop_nrt_profile.argtypes = [ctypes.c_char_p]
    lib.axon_stop_nrt_profile.restype = ctypes.c_int64

    @contextlib.contextmanager
    def _hook(output_dir, device_ids):
        import jax

        jax.devices()
        if device_ids:
            ids = (ctypes.c_int64 * len(device_ids))(*device_ids)
            rc = lib.axon_start_nrt_profile(ids, len(device_ids))
        else:
            rc = lib.axon_start_nrt_profile(None, 0)
        if rc != 0:
            raise RuntimeError(f"axon_start_nrt_profile rc={rc}")
        try:
            yield
        finally:
            n = lib.axon_stop_nrt_profile(str(output_dir).encode())
            print(f"profile: {n} file(s) written to {output_dir}", file=sys.stderr)

    mod.set_axon_ntff_profile_hook(_hook)


def run(inputs, trace=False):
    if trace:
        _install_profile_hook()
    in_maps, masked = _prep_in_maps(inputs)
    nc = _build(masked)
    res = bass_utils.run_bass_kernel_spmd(
        nc, in_maps, core_ids=list(range(NCORES)), trace=trace
    )
    out = np.zeros((B, S, D), np.float32)
    for c in range(NCORES):
        out[c // HG] += res.results[c]["out"]
    return out, res


def kernel(**inputs):
    return run(inputs, trace=False)[0]
